# revision 17
# baseline (speedup 1.0000x reference)
"""Trainium2 Bass kernel for nn_Encoder (KAN-style piecewise-linear MLP encoder).

Math: each adaptive piecewise-linear layer (P=3 knots on [-1,1]) collapses to
    out = u @ A + v @ C + bias,   u = clip(x,-1,1), v = clip(x,0,1)
with A = V1-V0, C = V0+V2-2*V1, bias = colsum(V1)  (hat basis sums to 1).
ELU never needs materializing: the next layer only consumes
    v' = clip(elu(h),0,1) = clip(h,0,1)
    u' = clip(elu(h),-1,1) + 1 = v' + exp(min(h,0))
and the +1 shift is folded into the next layer's bias (bias -= colsum(A_rows)).

v2 (this file): all activations/weights in bf16 (PE rate is unchanged vs
float32r, but DVE elementwise ops hit the 2x/4x fast modes and SBUF tiles
halve, which buys enough space to keep all 4 batch chunks in flight so the
tensor engine never drops out of its high p-state). LayerNorm rstd is
computed as exp(-0.5*ln(var+eps)) so every Activation-engine op (Identity
bias-add, Exp for ELU, Ln/Exp for rstd, Copy) lives in ONE act-func table
set - the baseline burned 22us reloading tables between Exp and Sqrt.
The -mean and rstd rows are broadcast across partitions with two K=1
matmuls, then copied once to SBUF bf16 so the LN apply (add+mul) runs in
DVE 2x mode. The last layer's bias is preloaded into PSUM with a K=1
matmul so the batch-major output needs only an Act Copy before DMA; the
output DMA issues from the idle GPSIMD DGE queue and the zs/act inputs
arrive as one large rearranged DMA each instead of 20 small ones.

Sharding: pure data-parallel, batch 16384 -> 8 x 2048. Activations are kept
feature-major ([feat, batch]) on chip; the host transposes + bf16-casts the
zs/action shards. LayerNorm stats (feature = partition axis) are computed
with 1/512-scaled ones-matmuls on the PE.

n_reps>1 wraps the computation in a hardware For-loop; used only by the
local timing harness to measure per-iteration device time by wall-clock
slope.
"""

import contextlib
import sys

sys.path.insert(0, "/opt/trn_rl_repo")

import numpy as np
import ml_dtypes

import concourse.bass as bass  # noqa: E402
import concourse.tile as tile  # noqa: E402
from concourse import bacc, mybir  # noqa: E402
from concourse.bass_utils import run_bass_kernel_spmd  # noqa: E402

F32 = mybir.dt.float32
BF16 = mybir.dt.bfloat16
BF = ml_dtypes.bfloat16
AF = mybir.ActivationFunctionType
OP = mybir.AluOpType

NCORES = 8
B_LOC = 2048          # batch rows per core
BC = 512              # batch columns per chunk (psum free dim)
NB = B_LOC // BC      # 4 batch chunks
P = 128
LN_EPS = 1e-5


def build_module(n_reps=1):
    nc = bacc.Bacc("TRN2", target_bir_lowering=False, debug=False,
                   enable_asserts=False, num_devices=NCORES)

    def din(name, shape, dt=BF16):
        return nc.dram_tensor(name, list(shape), dt, kind="ExternalInput").ap()

    zsT = din("zsT", (512, B_LOC))
    actT = din("actT", (8, B_LOC))
    wza = din("wza", (8, 2, 256))
    w1 = din("w1", (1536, 512))
    w2 = din("w2", (1024, 512))
    w3 = din("w3", (1024, 512))
    bza_p = din("bza_p", (128, 2), F32)
    b1_p = din("b1_p", (128, 4), F32)
    b2_p = din("b2_p", (128, 4), F32)
    b3_r = din("b3_r", (1, 512))
    ones_c = din("ones_c", (1, 128))
    oinv_m = din("oinv_m", (128, 1))
    out = nc.dram_tensor("out", [B_LOC, 512], F32, kind="ExternalOutput").ap()

    with tile.TileContext(nc) as tc:
        with (
            tc.tile_pool(name="wpool", bufs=1) as wp,
            tc.tile_pool(name="io", bufs=1) as io,
            tc.tile_pool(name="uvza", bufs=2) as uvza,
            tc.tile_pool(name="uvzs", bufs=3) as uvzs,
            tc.tile_pool(name="uv1", bufs=2) as uv1,
            tc.tile_pool(name="uv2", bufs=2) as uv2,
            tc.tile_pool(name="zcb", bufs=2) as zcbp,
            tc.tile_pool(name="eph", bufs=2) as eph,
            tc.tile_pool(name="rows", bufs=2) as rows,
            tc.tile_pool(name="bcsb", bufs=2) as bcsb,
            tc.tile_pool(name="osb", bufs=2) as osbp,
            tc.tile_pool(name="psz", bufs=4, space="PSUM") as psz,
            tc.tile_pool(name="psst", bufs=2, space="PSUM") as psst,
            tc.tile_pool(name="psbc", bufs=1, space="PSUM") as psbc,
        ):
            # ---- persistent weights / constants ----
            w1_sb = wp.tile([P, 12, 512], BF16)
            nc.sync.dma_start(w1_sb[:], w1.rearrange("(c p) o -> p c o", p=P))
            w2_sb = wp.tile([P, 8, 512], BF16)
            nc.sync.dma_start(w2_sb[:], w2.rearrange("(c p) o -> p c o", p=P))
            w3_sb = wp.tile([P, 8, 512], BF16)
            nc.sync.dma_start(w3_sb[:], w3.rearrange("(c p) o -> p c o", p=P))
            wza_sb = wp.tile([8, 2, 256], BF16)
            nc.sync.dma_start(wza_sb[:], wza[:, :, :])
            bza_sb = wp.tile([P, 2], F32)
            nc.sync.dma_start(bza_sb[:], bza_p[:, :])
            b1_sb = wp.tile([P, 4], F32)
            nc.sync.dma_start(b1_sb[:], b1_p[:, :])
            b2_sb = wp.tile([P, 4], F32)
            nc.sync.dma_start(b2_sb[:], b2_p[:, :])
            b3_sb = wp.tile([1, 512], BF16)
            nc.sync.dma_start(b3_sb[:], b3_r[:, :])
            ones_col = wp.tile([1, 128], BF16)
            nc.sync.dma_start(ones_col[:], ones_c[:, :])
            oinv_mcol = wp.tile([P, 1], BF16)  # 1/512 -> stats matmuls -> means
            nc.sync.dma_start(oinv_mcol[:], oinv_m[:, :])
            eps_sb = wp.tile([1, 1], F32)
            nc.vector.memset(eps_sb[:], LN_EPS)

            def body():
                # ---- per-chunk persistent activation tiles (bf16) ----
                upza = [uvza.tile([P, 2, BC], BF16, tag="upza", name=f"upza{b}")
                        for b in range(NB)]
                vza = [uvza.tile([P, 2, BC], BF16, tag="vza", name=f"vza{b}")
                       for b in range(NB)]
                up1 = [uv1.tile([P, 4, BC], BF16, tag="up1", name=f"up1{b}")
                       for b in range(NB)]
                v1 = [uv1.tile([P, 4, BC], BF16, tag="v1", name=f"v1{b}")
                      for b in range(NB)]
                up2 = [uv2.tile([P, 4, BC], BF16, tag="up2", name=f"up2{b}")
                       for b in range(NB)]
                v2 = [uv2.tile([P, 4, BC], BF16, tag="v2", name=f"v2{b}")
                      for b in range(NB)]

                # ==== consolidated input DMA + clips (single big transfers)
                act_raw = eph.tile([8, B_LOC], BF16, tag="act_raw", bufs=1)
                nc.sync.dma_start(act_raw[:], actT[:, :])
                u_act = eph.tile([8, B_LOC], BF16, tag="u_act", bufs=1)
                nc.vector.tensor_scalar(u_act[:], act_raw[:],
                                        -1.0, 1.0, OP.max, OP.min)
                v_act = eph.tile([8, B_LOC], BF16, tag="v_act", bufs=1)
                nc.vector.tensor_scalar(v_act[:], act_raw[:],
                                        0.0, 1.0, OP.max, OP.min)
                zsraw = io.tile([P, 4, B_LOC], BF16, tag="zsraw")
                nc.sync.dma_start(zsraw[:],
                                  zsT.rearrange("(c p) b -> p c b", p=P))
                uzs, vzs = [], []
                for b in range(NB):
                    bs = slice(b * BC, (b + 1) * BC)
                    u = uvzs.tile([P, 4, BC], BF16, tag="uzs", name=f"uzs{b}")
                    nc.vector.tensor_scalar(u[:], zsraw[:, :, bs],
                                            -1.0, 1.0, OP.max, OP.min)
                    v = uvzs.tile([P, 4, BC], BF16, tag="vzs", name=f"vzs{b}")
                    nc.vector.tensor_scalar(v[:], zsraw[:, :, bs],
                                            0.0, 1.0, OP.max, OP.min)
                    uzs.append(u)
                    vzs.append(v)

                # ==== l0: za = (pre-elu) APL(action) ====
                for b in range(NB):
                    bs = slice(b * BC, (b + 1) * BC)
                    zb = zcbp.tile([P, 2, BC], BF16, tag="zb_za")
                    for o in range(2):
                        zps = psz.tile([P, BC], F32, tag="z")
                        nc.tensor.matmul(zps[:], wza_sb[:, 0, bass.ts(o, 128)],
                                         u_act[:, bs], start=True, stop=False)
                        nc.tensor.matmul(zps[:], wza_sb[:, 1, bass.ts(o, 128)],
                                         v_act[:, bs], start=False, stop=True)
                        nc.scalar.activation(zb[:, o, :], zps[:], AF.Identity,
                                             bias=bza_sb[:, o:o + 1])
                    nc.vector.tensor_scalar(vza[b][:], zb[:], 0.0, 1.0,
                                            OP.max, OP.min)
                    nmin = eph.tile([P, 2, BC], BF16, tag="nmin_za")
                    nc.vector.tensor_scalar(nmin[:], zb[:], 0.0, None, OP.min)
                    nc.scalar.activation(nmin[:], nmin[:], AF.Exp)
                    nc.vector.tensor_add(upza[b][:], vza[b][:], nmin[:])

                # ==== hidden layers l1, l2 ====
                def hidden_layer(b, KC, w_sb, b_sb, rhs_fn, up_dst, v_dst):
                    zc = zcbp.tile([P, 4, BC], BF16, tag="zcb")
                    for o in range(4):
                        zps = psz.tile([P, BC], F32, tag="z")
                        for k in range(KC):
                            nc.tensor.matmul(zps[:],
                                             w_sb[:, k, bass.ts(o, 128)],
                                             rhs_fn(k),
                                             start=(k == 0),
                                             stop=(k == KC - 1))
                        nc.scalar.activation(zc[:, o, :], zps[:],
                                             AF.Identity,
                                             bias=b_sb[:, o:o + 1])
                    # stats: mean and mean-square via 1/512-ones matmuls (M=1)
                    st_ps = psst.tile([33, BC], F32, tag="st")
                    for o in range(4):
                        nc.tensor.matmul(st_ps[0:1, :], oinv_mcol[:],
                                         zc[:, o, :],
                                         start=(o == 0), stop=(o == 3))
                    zsq = eph.tile([P, 4, BC], BF16, tag="zsq")
                    nc.vector.tensor_mul(zsq[:], zc[:], zc[:])
                    for o in range(4):
                        nc.tensor.matmul(st_ps[32:33, :], oinv_mcol[:],
                                         zsq[:, o, :],
                                         start=(o == 0), stop=(o == 3))
                    # rows: cat = [-mean | rstd] as one [1, 2*BC] bf16 row
                    cat = rows.tile([1, 2, BC], BF16, tag="cat")
                    nc.vector.tensor_scalar(cat[0:1, 0, :], st_ps[0:1, :],
                                            -1.0, None, OP.mult)
                    m2 = rows.tile([1, BC], F32, tag="scr", name="m2")
                    nc.scalar.activation(m2[:], st_ps[0:1, :], AF.Square)
                    var = rows.tile([1, BC], F32, tag="scr", name="var")
                    nc.vector.tensor_sub(var[:], st_ps[32:33, :], m2[:])
                    lnv = rows.tile([1, BC], F32, tag="scr", name="lnv")
                    nc.scalar.activation(lnv[:], var[:], AF.Ln, bias=eps_sb[:])
                    with nc.allow_low_precision(
                            reason="rstd rounds to bf16 for PE broadcast"):
                        nc.scalar.activation(cat[0:1, 1, :], lnv[:], AF.Exp,
                                             scale=-0.5)
                    # broadcast [-m | s] across partitions (one K=1 matmul)
                    bc_ps = psbc.tile([P, 2, BC], F32, tag="bc")
                    nc.tensor.matmul(bc_ps[:, 0, :], ones_col[:],
                                     cat[0:1, 0, :], start=True, stop=True)
                    nc.tensor.matmul(bc_ps[:, 1, :], ones_col[:],
                                     cat[0:1, 1, :], start=True, stop=True)
                    bc_sb = bcsb.tile([P, 2, BC], BF16, tag="bc_sb")
                    nc.scalar.activation(bc_sb[:], bc_ps[:], AF.Copy)
                    mb_b = bc_sb[:, 0:1, :].to_broadcast([P, 4, BC])
                    sb_b = bc_sb[:, 1:2, :].to_broadcast([P, 4, BC])
                    h = zc
                    nc.vector.tensor_add(h[:], h[:], mb_b)
                    nc.vector.tensor_mul(h[:], h[:], sb_b)
                    nc.vector.tensor_scalar(v_dst[:], h[:],
                                            0.0, 1.0, OP.max, OP.min)
                    nmin = eph.tile([P, 4, BC], BF16, tag="nmin")
                    nc.vector.tensor_scalar(nmin[:], h[:], 0.0, None, OP.min)
                    nc.scalar.activation(nmin[:], nmin[:], AF.Exp)
                    nc.vector.tensor_add(up_dst[:], v_dst[:], nmin[:])

                for b in range(NB):
                    def rhs1(k, b=b):
                        if k < 4:
                            return uzs[b][:, k, :]
                        if k < 8:
                            return vzs[b][:, k - 4, :]
                        if k < 10:
                            return upza[b][:, k - 8, :]
                        return vza[b][:, k - 10, :]

                    hidden_layer(b, 12, w1_sb, b1_sb, rhs1, up1[b], v1[b])

                for b in range(NB):
                    def rhs2(k, b=b):
                        return up1[b][:, k, :] if k < 4 else v1[b][:, k - 4, :]

                    hidden_layer(b, 8, w2_sb, b2_sb, rhs2, up2[b], v2[b])

                # ==== l3: batch-major out, bias preloaded via K=1 matmul ====
                for b in range(NB):
                    osb = osbp.tile([P, 4, 512], F32, tag="osb")
                    for q in range(4):
                        qs = bass.ts(q, 128)
                        ops = psz.tile([P, 512], F32, tag="z")
                        nc.tensor.matmul(ops[:], ones_col[:], b3_sb[:],
                                         start=True, stop=False)
                        for k in range(8):
                            lhsT = (up2[b][:, k, qs] if k < 4
                                    else v2[b][:, k - 4, qs])
                            nc.tensor.matmul(ops[:], lhsT, w3_sb[:, k, :],
                                             start=False, stop=(k == 7))
                        nc.scalar.activation(osb[:, q, :], ops[:], AF.Copy)
                    nc.gpsimd.dma_start(
                        out[b * BC:(b + 1) * BC, :].rearrange(
                            "(q p) n -> p q n", p=P),
                        osb[:])

            rep_ctx = (tc.For_i(0, n_reps, 1) if n_reps > 1
                       else contextlib.nullcontext())
            with rep_ctx:
                body()

    # The act-table pass picks the FIRST set containing each function, so
    # Ln (set 5) and Exp (set 0) alternate and every hidden-layer chunk
    # pays two 1283ns table loads.  All functions this kernel uses
    # (Identity, Square, Ln, Exp, Copy) live together in
    # 'natural_log_exp_and_others' (set 6): censor sets 0-5 in the copy the
    # pass sees so every function first-matches at set 6 -> ONE load total.
    # Only the pass's choice is patched; emitted ids still index the real
    # act_info.json list, and set 6's real contents cover every function.
    import concourse.bacc as _bacc_mod
    _orig_tables = _bacc_mod.get_activation_tables
    _used = {AF.Identity, AF.Square, AF.Ln, AF.Exp, AF.Copy}

    def _censored_tables(arch):
        tabs = dict(_orig_tables(arch))
        names = list(tabs)
        for name in names[:6]:
            tabs[name] = tabs[name] - _used
        return tabs

    _bacc_mod.get_activation_tables = _censored_tables
    try:
        nc.compile()
    finally:
        _bacc_mod.get_activation_tables = _orig_tables
    return nc


def fold_weights(W_za, W1, W2, W3):
    def fold(vals):
        V = vals.astype(np.float64)
        A = V[:, :, 1] - V[:, :, 0]
        C = V[:, :, 0] + V[:, :, 2] - 2.0 * V[:, :, 1]
        b = V[:, :, 1].sum(axis=0)
        return A, C, b

    A0, C0, b0 = fold(W_za)
    A1, C1, b1 = fold(W1)
    A2, C2, b2 = fold(W2)
    A3, C3, b3 = fold(W3)

    wza = np.stack([A0, C0], axis=1)                             # [8, 2, 256]
    w1 = np.concatenate([A1[:512], C1[:512], A1[512:], C1[512:]], axis=0)
    w2 = np.concatenate([A2, C2], axis=0)                        # [1024, 512]
    w3 = np.concatenate([A3, C3], axis=0)                        # [1024, 512]
    b1e = b1 - A1[512:].sum(axis=0)      # za u' carries +1 shift
    b2e = b2 - A2.sum(axis=0)
    b3e = b3 - A3.sum(axis=0)

    f = np.float32
    return {
        "wza": np.ascontiguousarray(wza.astype(f), BF),
        "w1": np.ascontiguousarray(w1.astype(f), BF),
        "w2": np.ascontiguousarray(w2.astype(f), BF),
        "w3": np.ascontiguousarray(w3.astype(f), BF),
        "bza_p": np.ascontiguousarray(b0.reshape(2, 128).T, f),
        "b1_p": np.ascontiguousarray(b1e.reshape(4, 128).T, f),
        "b2_p": np.ascontiguousarray(b2e.reshape(4, 128).T, f),
        "b3_r": np.ascontiguousarray(b3e.reshape(1, 512).astype(f), BF),
        "ones_c": np.ones((1, 128), BF),
        "oinv_m": np.full((128, 1), 1.0 / 512.0, BF),
    }


_NC_CACHE = {}


def get_module(n_reps=1):
    key = f"nc{n_reps}"
    if key not in _NC_CACHE:
        _NC_CACHE[key] = build_module(n_reps)
    return _NC_CACHE[key]


def make_in_maps(zs, action, W_za, W1, W2, W3):
    wmap = fold_weights(np.asarray(W_za), np.asarray(W1), np.asarray(W2),
                        np.asarray(W3))
    in_maps = []
    for c in range(NCORES):
        sl = slice(c * B_LOC, (c + 1) * B_LOC)
        m = dict(wmap)
        m["zsT"] = np.ascontiguousarray(np.asarray(zs)[sl].T.astype(np.float32),
                                        BF)
        m["actT"] = np.ascontiguousarray(
            np.asarray(action)[sl].T.astype(np.float32), BF)
        in_maps.append(m)
    return in_maps


def kernel(zs, action, W_za, W1, W2, W3, _trace=False, _tmpdir=None):
    nc = get_module()
    in_maps = make_in_maps(zs, action, W_za, W1, W2, W3)
    res = run_bass_kernel_spmd(nc, in_maps, core_ids=list(range(NCORES)),
                               trace=_trace, tmpdir=_tmpdir)
    out = np.concatenate([res.results[c]["out"] for c in range(NCORES)],
                         axis=0).astype(np.float32)
    if _trace:
        kernel.last_exec_time_ns = res.exec_time_ns
        kernel.last_results = res
    return out


# revision 18
# speedup vs baseline: 1.0656x; 1.0656x over previous
"""Trainium2 Bass kernel for nn_Encoder (KAN-style piecewise-linear MLP encoder).

Math: each adaptive piecewise-linear layer (P=3 knots on [-1,1]) collapses to
    out = u @ A + v @ C + bias,   u = clip(x,-1,1), v = clip(x,0,1)
with A = V1-V0, C = V0+V2-2*V1, bias = colsum(V1)  (hat basis sums to 1).
ELU never needs materializing: the next layer only consumes
    v' = clip(elu(h),0,1) = clip(h,0,1)
    u' = clip(elu(h),-1,1) + 1 = v' + exp(min(h,0))
and the +1 shift is folded into the next layer's bias (bias -= colsum(A_rows)).

v2 (this file): all activations/weights in bf16 (PE rate is unchanged vs
float32r, but DVE elementwise ops hit the 2x/4x fast modes and SBUF tiles
halve, which buys enough space to keep all 4 batch chunks in flight so the
tensor engine never drops out of its high p-state). LayerNorm rstd is
computed as exp(-0.5*ln(var+eps)) so every Activation-engine op (Identity
bias-add, Exp for ELU, Ln/Exp for rstd, Copy) lives in ONE act-func table
set - the baseline burned 22us reloading tables between Exp and Sqrt.
The -mean and rstd rows are broadcast across partitions with two K=1
matmuls, then copied once to SBUF bf16 so the LN apply (add+mul) runs in
DVE 2x mode. The last layer's bias is preloaded into PSUM with a K=1
matmul so the batch-major output needs only an Act Copy before DMA; the
output DMA issues from the idle GPSIMD DGE queue and the zs/act inputs
arrive as one large rearranged DMA each instead of 20 small ones.

Sharding: pure data-parallel, batch 16384 -> 8 x 2048. Activations are kept
feature-major ([feat, batch]) on chip; the host transposes + bf16-casts the
zs/action shards. LayerNorm stats (feature = partition axis) are computed
with 1/512-scaled ones-matmuls on the PE.

n_reps>1 wraps the computation in a hardware For-loop; used only by the
local timing harness to measure per-iteration device time by wall-clock
slope.
"""

import contextlib
import sys

sys.path.insert(0, "/opt/trn_rl_repo")

import numpy as np
import ml_dtypes

import concourse.bass as bass  # noqa: E402
import concourse.tile as tile  # noqa: E402
from concourse import bacc, mybir  # noqa: E402
from concourse.bass_utils import run_bass_kernel_spmd  # noqa: E402

F32 = mybir.dt.float32
BF16 = mybir.dt.bfloat16
BF = ml_dtypes.bfloat16
AF = mybir.ActivationFunctionType
OP = mybir.AluOpType

NCORES = 8
B_LOC = 2048          # batch rows per core
BC = 512              # batch columns per chunk (psum free dim)
NB = B_LOC // BC      # 4 batch chunks
P = 128
LN_EPS = 1e-5


def build_module(n_reps=1):
    nc = bacc.Bacc("TRN2", target_bir_lowering=False, debug=False,
                   enable_asserts=False, num_devices=NCORES)

    def din(name, shape, dt=BF16):
        return nc.dram_tensor(name, list(shape), dt, kind="ExternalInput").ap()

    zsT = din("zsT", (512, B_LOC))
    actT = din("actT", (8, B_LOC))
    wza = din("wza", (8, 2, 256))
    w1 = din("w1", (1536, 512))
    w2 = din("w2", (1024, 512))
    w3 = din("w3", (1024, 512))
    bza_p = din("bza_p", (128, 2), F32)
    b1_p = din("b1_p", (128, 4), F32)
    b2_p = din("b2_p", (128, 4), F32)
    b3_r = din("b3_r", (1, 512))
    ones_c = din("ones_c", (1, 128))
    oinv_m = din("oinv_m", (128, 1))
    out = nc.dram_tensor("out", [B_LOC, 512], F32, kind="ExternalOutput").ap()

    with tile.TileContext(nc) as tc:
        with (
            tc.tile_pool(name="wpool", bufs=1) as wp,
            tc.tile_pool(name="io", bufs=1) as io,
            tc.tile_pool(name="uvza", bufs=2) as uvza,
            tc.tile_pool(name="uvzs", bufs=3) as uvzs,
            tc.tile_pool(name="uv1", bufs=2) as uv1,
            tc.tile_pool(name="uv2", bufs=2) as uv2,
            tc.tile_pool(name="zcb", bufs=3) as zcbp,
            tc.tile_pool(name="eph", bufs=2) as eph,
            tc.tile_pool(name="rows", bufs=2) as rows,
            tc.tile_pool(name="bcsb", bufs=2) as bcsb,
            tc.tile_pool(name="osb", bufs=2) as osbp,
            tc.tile_pool(name="psz", bufs=4, space="PSUM") as psz,
            tc.tile_pool(name="psst", bufs=2, space="PSUM") as psst,
            tc.tile_pool(name="psbc", bufs=1, space="PSUM") as psbc,
        ):
            # ---- persistent weights / constants ----
            w1_sb = wp.tile([P, 12, 512], BF16)
            nc.sync.dma_start(w1_sb[:], w1.rearrange("(c p) o -> p c o", p=P))
            w2_sb = wp.tile([P, 8, 512], BF16)
            nc.sync.dma_start(w2_sb[:], w2.rearrange("(c p) o -> p c o", p=P))
            w3_sb = wp.tile([P, 8, 512], BF16)
            nc.sync.dma_start(w3_sb[:], w3.rearrange("(c p) o -> p c o", p=P))
            wza_sb = wp.tile([8, 2, 256], BF16)
            nc.sync.dma_start(wza_sb[:], wza[:, :, :])
            bza_sb = wp.tile([P, 2], F32)
            nc.sync.dma_start(bza_sb[:], bza_p[:, :])
            b1_sb = wp.tile([P, 4], F32)
            nc.sync.dma_start(b1_sb[:], b1_p[:, :])
            b2_sb = wp.tile([P, 4], F32)
            nc.sync.dma_start(b2_sb[:], b2_p[:, :])
            b3_sb = wp.tile([1, 512], BF16)
            nc.sync.dma_start(b3_sb[:], b3_r[:, :])
            ones_col = wp.tile([1, 128], BF16)
            nc.sync.dma_start(ones_col[:], ones_c[:, :])
            oinv_mcol = wp.tile([P, 1], BF16)  # 1/512 -> stats matmuls -> means
            nc.sync.dma_start(oinv_mcol[:], oinv_m[:, :])
            eps_sb = wp.tile([1, 1], F32)
            nc.vector.memset(eps_sb[:], LN_EPS)

            def body():
                # ---- per-chunk persistent activation tiles (bf16) ----
                upza = [uvza.tile([P, 2, BC], BF16, tag="upza", name=f"upza{b}")
                        for b in range(NB)]
                vza = [uvza.tile([P, 2, BC], BF16, tag="vza", name=f"vza{b}")
                       for b in range(NB)]
                up1 = [uv1.tile([P, 4, BC], BF16, tag="up1", name=f"up1{b}")
                       for b in range(NB)]
                v1 = [uv1.tile([P, 4, BC], BF16, tag="v1", name=f"v1{b}")
                      for b in range(NB)]
                up2 = [uv2.tile([P, 4, BC], BF16, tag="up2", name=f"up2{b}")
                       for b in range(NB)]
                v2 = [uv2.tile([P, 4, BC], BF16, tag="v2", name=f"v2{b}")
                      for b in range(NB)]

                # ==== consolidated input DMA + clips (single big transfers)
                act_raw = eph.tile([8, B_LOC], BF16, tag="act_raw", bufs=1)
                nc.sync.dma_start(act_raw[:], actT[:, :])
                u_act = eph.tile([8, B_LOC], BF16, tag="u_act", bufs=1)
                nc.vector.tensor_scalar(u_act[:], act_raw[:],
                                        -1.0, 1.0, OP.max, OP.min)
                v_act = eph.tile([8, B_LOC], BF16, tag="v_act", bufs=1)
                nc.vector.tensor_scalar(v_act[:], act_raw[:],
                                        0.0, 1.0, OP.max, OP.min)
                zsraw = io.tile([P, 4, B_LOC], BF16, tag="zsraw")
                nc.sync.dma_start(zsraw[:],
                                  zsT.rearrange("(c p) b -> p c b", p=P))
                uzs, vzs = [], []
                for b in range(NB):
                    bs = slice(b * BC, (b + 1) * BC)
                    u = uvzs.tile([P, 4, BC], BF16, tag="uzs", name=f"uzs{b}")
                    nc.vector.tensor_scalar(u[:], zsraw[:, :, bs],
                                            -1.0, 1.0, OP.max, OP.min)
                    v = uvzs.tile([P, 4, BC], BF16, tag="vzs", name=f"vzs{b}")
                    nc.vector.tensor_scalar(v[:], zsraw[:, :, bs],
                                            0.0, 1.0, OP.max, OP.min)
                    uzs.append(u)
                    vzs.append(v)

                # ==== l0: za = (pre-elu) APL(action) ====
                for b in range(NB):
                    bs = slice(b * BC, (b + 1) * BC)
                    zb = zcbp.tile([P, 2, BC], BF16, tag="zb_za")
                    for o in range(2):
                        zps = psz.tile([P, BC], F32, tag="z")
                        nc.tensor.matmul(zps[:], wza_sb[:, 0, bass.ts(o, 128)],
                                         u_act[:, bs], start=True, stop=False)
                        nc.tensor.matmul(zps[:], wza_sb[:, 1, bass.ts(o, 128)],
                                         v_act[:, bs], start=False, stop=True)
                        nc.scalar.activation(zb[:, o, :], zps[:], AF.Identity,
                                             bias=bza_sb[:, o:o + 1])
                    nc.vector.tensor_scalar(vza[b][:], zb[:], 0.0, 1.0,
                                            OP.max, OP.min)
                    nmin = eph.tile([P, 2, BC], BF16, tag="nmin_za")
                    nc.vector.tensor_scalar(nmin[:], zb[:], 0.0, None, OP.min)
                    nc.scalar.activation(nmin[:], nmin[:], AF.Exp)
                    nc.vector.tensor_add(upza[b][:], vza[b][:], nmin[:])

                # ==== hidden layers l1, l2 ====
                def hidden_layer(b, KC, w_sb, b_sb, rhs_fn, up_dst, v_dst):
                    zc = zcbp.tile([P, 4, BC], BF16, tag="zcb")
                    for o in range(4):
                        zps = psz.tile([P, BC], F32, tag="z")
                        for k in range(KC):
                            nc.tensor.matmul(zps[:],
                                             w_sb[:, k, bass.ts(o, 128)],
                                             rhs_fn(k),
                                             start=(k == 0),
                                             stop=(k == KC - 1))
                        nc.scalar.activation(zc[:, o, :], zps[:],
                                             AF.Identity,
                                             bias=b_sb[:, o:o + 1])
                    # stats: mean and mean-square via 1/512-ones matmuls (M=1)
                    st_ps = psst.tile([33, BC], F32, tag="st")
                    for o in range(4):
                        nc.tensor.matmul(st_ps[0:1, :], oinv_mcol[:],
                                         zc[:, o, :],
                                         start=(o == 0), stop=(o == 3))
                    zsq = eph.tile([P, 4, BC], BF16, tag="zsq")
                    nc.vector.tensor_mul(zsq[:], zc[:], zc[:])
                    for o in range(4):
                        nc.tensor.matmul(st_ps[32:33, :], oinv_mcol[:],
                                         zsq[:, o, :],
                                         start=(o == 0), stop=(o == 3))
                    # rows: cat = [-mean | rstd] as one [1, 2*BC] bf16 row
                    cat = rows.tile([1, 2, BC], BF16, tag="cat")
                    nc.vector.tensor_scalar(cat[0:1, 0, :], st_ps[0:1, :],
                                            -1.0, None, OP.mult)
                    m2 = rows.tile([1, BC], F32, tag="scr", name="m2")
                    nc.scalar.activation(m2[:], st_ps[0:1, :], AF.Square)
                    var = rows.tile([1, BC], F32, tag="scr", name="var")
                    nc.vector.tensor_sub(var[:], st_ps[32:33, :], m2[:])
                    lnv = rows.tile([1, BC], F32, tag="scr", name="lnv")
                    nc.scalar.activation(lnv[:], var[:], AF.Ln, bias=eps_sb[:])
                    with nc.allow_low_precision(
                            reason="rstd rounds to bf16 for PE broadcast"):
                        nc.scalar.activation(cat[0:1, 1, :], lnv[:], AF.Exp,
                                             scale=-0.5)
                    # broadcast [-m | s] across partitions (one K=1 matmul)
                    bc_ps = psbc.tile([P, 2, BC], F32, tag="bc")
                    nc.tensor.matmul(bc_ps[:, 0, :], ones_col[:],
                                     cat[0:1, 0, :], start=True, stop=True)
                    nc.tensor.matmul(bc_ps[:, 1, :], ones_col[:],
                                     cat[0:1, 1, :], start=True, stop=True)
                    bc_sb = bcsb.tile([P, 2, BC], BF16, tag="bc_sb")
                    nc.scalar.activation(bc_sb[:], bc_ps[:], AF.Copy)
                    mb_b = bc_sb[:, 0:1, :].to_broadcast([P, 4, BC])
                    sb_b = bc_sb[:, 1:2, :].to_broadcast([P, 4, BC])
                    h = zc
                    nc.vector.tensor_add(h[:], h[:], mb_b)
                    nc.vector.tensor_mul(h[:], h[:], sb_b)
                    nc.vector.tensor_scalar(v_dst[:], h[:],
                                            0.0, 1.0, OP.max, OP.min)
                    nmin = eph.tile([P, 4, BC], BF16, tag="nmin")
                    nc.vector.tensor_scalar(nmin[:], h[:], 0.0, None, OP.min)
                    nc.scalar.activation(nmin[:], nmin[:], AF.Exp)
                    nc.vector.tensor_add(up_dst[:], v_dst[:], nmin[:])

                for b in range(NB):
                    def rhs1(k, b=b):
                        if k < 4:
                            return uzs[b][:, k, :]
                        if k < 8:
                            return vzs[b][:, k - 4, :]
                        if k < 10:
                            return upza[b][:, k - 8, :]
                        return vza[b][:, k - 10, :]

                    hidden_layer(b, 12, w1_sb, b1_sb, rhs1, up1[b], v1[b])

                for b in range(NB):
                    def rhs2(k, b=b):
                        return up1[b][:, k, :] if k < 4 else v1[b][:, k - 4, :]

                    hidden_layer(b, 8, w2_sb, b2_sb, rhs2, up2[b], v2[b])

                # ==== l3: batch-major out, bias preloaded via K=1 matmul ====
                for b in range(NB):
                    for h in range(2):
                        osb = osbp.tile([P, 2, 512], F32, tag="osb")
                        for qq in range(2):
                            q = h * 2 + qq
                            qs = bass.ts(q, 128)
                            ops = psz.tile([P, 512], F32, tag="z")
                            nc.tensor.matmul(ops[:], ones_col[:], b3_sb[:],
                                             start=True, stop=False)
                            for k in range(8):
                                lhsT = (up2[b][:, k, qs] if k < 4
                                        else v2[b][:, k - 4, qs])
                                nc.tensor.matmul(ops[:], lhsT, w3_sb[:, k, :],
                                                 start=False, stop=(k == 7))
                            nc.scalar.activation(osb[:, qq, :], ops[:],
                                                 AF.Copy)
                        nc.gpsimd.dma_start(
                            out[b * BC + h * 256:
                                b * BC + (h + 1) * 256, :].rearrange(
                                "(q p) n -> p q n", p=P),
                            osb[:])

            rep_ctx = (tc.For_i(0, n_reps, 1) if n_reps > 1
                       else contextlib.nullcontext())
            with rep_ctx:
                body()

    # The act-table pass picks the FIRST set containing each function, so
    # Ln (set 5) and Exp (set 0) alternate and every hidden-layer chunk
    # pays two 1283ns table loads.  All functions this kernel uses
    # (Identity, Square, Ln, Exp, Copy) live together in
    # 'natural_log_exp_and_others' (set 6): censor sets 0-5 in the copy the
    # pass sees so every function first-matches at set 6 -> ONE load total.
    # Only the pass's choice is patched; emitted ids still index the real
    # act_info.json list, and set 6's real contents cover every function.
    import concourse.bacc as _bacc_mod
    _orig_tables = _bacc_mod.get_activation_tables
    _used = {AF.Identity, AF.Square, AF.Ln, AF.Exp, AF.Copy}

    def _censored_tables(arch):
        tabs = dict(_orig_tables(arch))
        names = list(tabs)
        for name in names[:6]:
            tabs[name] = tabs[name] - _used
        return tabs

    _bacc_mod.get_activation_tables = _censored_tables
    try:
        nc.compile()
    finally:
        _bacc_mod.get_activation_tables = _orig_tables
    return nc


def fold_weights(W_za, W1, W2, W3):
    def fold(vals):
        V = vals.astype(np.float64)
        A = V[:, :, 1] - V[:, :, 0]
        C = V[:, :, 0] + V[:, :, 2] - 2.0 * V[:, :, 1]
        b = V[:, :, 1].sum(axis=0)
        return A, C, b

    A0, C0, b0 = fold(W_za)
    A1, C1, b1 = fold(W1)
    A2, C2, b2 = fold(W2)
    A3, C3, b3 = fold(W3)

    wza = np.stack([A0, C0], axis=1)                             # [8, 2, 256]
    w1 = np.concatenate([A1[:512], C1[:512], A1[512:], C1[512:]], axis=0)
    w2 = np.concatenate([A2, C2], axis=0)                        # [1024, 512]
    w3 = np.concatenate([A3, C3], axis=0)                        # [1024, 512]
    b1e = b1 - A1[512:].sum(axis=0)      # za u' carries +1 shift
    b2e = b2 - A2.sum(axis=0)
    b3e = b3 - A3.sum(axis=0)

    f = np.float32
    return {
        "wza": np.ascontiguousarray(wza.astype(f), BF),
        "w1": np.ascontiguousarray(w1.astype(f), BF),
        "w2": np.ascontiguousarray(w2.astype(f), BF),
        "w3": np.ascontiguousarray(w3.astype(f), BF),
        "bza_p": np.ascontiguousarray(b0.reshape(2, 128).T, f),
        "b1_p": np.ascontiguousarray(b1e.reshape(4, 128).T, f),
        "b2_p": np.ascontiguousarray(b2e.reshape(4, 128).T, f),
        "b3_r": np.ascontiguousarray(b3e.reshape(1, 512).astype(f), BF),
        "ones_c": np.ones((1, 128), BF),
        "oinv_m": np.full((128, 1), 1.0 / 512.0, BF),
    }


_NC_CACHE = {}


def get_module(n_reps=1):
    key = f"nc{n_reps}"
    if key not in _NC_CACHE:
        _NC_CACHE[key] = build_module(n_reps)
    return _NC_CACHE[key]


def make_in_maps(zs, action, W_za, W1, W2, W3):
    wmap = fold_weights(np.asarray(W_za), np.asarray(W1), np.asarray(W2),
                        np.asarray(W3))
    in_maps = []
    for c in range(NCORES):
        sl = slice(c * B_LOC, (c + 1) * B_LOC)
        m = dict(wmap)
        m["zsT"] = np.ascontiguousarray(np.asarray(zs)[sl].T.astype(np.float32),
                                        BF)
        m["actT"] = np.ascontiguousarray(
            np.asarray(action)[sl].T.astype(np.float32), BF)
        in_maps.append(m)
    return in_maps


def kernel(zs, action, W_za, W1, W2, W3, _trace=False, _tmpdir=None):
    nc = get_module()
    in_maps = make_in_maps(zs, action, W_za, W1, W2, W3)
    res = run_bass_kernel_spmd(nc, in_maps, core_ids=list(range(NCORES)),
                               trace=_trace, tmpdir=_tmpdir)
    out = np.concatenate([res.results[c]["out"] for c in range(NCORES)],
                         axis=0).astype(np.float32)
    if _trace:
        kernel.last_exec_time_ns = res.exec_time_ns
        kernel.last_results = res
    return out


# revision 19
# speedup vs baseline: 1.0748x; 1.0086x over previous
"""Trainium2 Bass kernel for nn_Encoder (KAN-style piecewise-linear MLP encoder).

Math: each adaptive piecewise-linear layer (P=3 knots on [-1,1]) collapses to
    out = u @ A + v @ C + bias,   u = clip(x,-1,1), v = clip(x,0,1)
with A = V1-V0, C = V0+V2-2*V1, bias = colsum(V1)  (hat basis sums to 1).
ELU never needs materializing: the next layer only consumes
    v' = clip(elu(h),0,1) = clip(h,0,1)
    u' = clip(elu(h),-1,1) + 1 = v' + exp(min(h,0))
and the +1 shift is folded into the next layer's bias (bias -= colsum(A_rows)).

v2 (this file): all activations/weights in bf16 (PE rate is unchanged vs
float32r, but DVE elementwise ops hit the 2x/4x fast modes and SBUF tiles
halve, which buys enough space to keep all 4 batch chunks in flight so the
tensor engine never drops out of its high p-state). LayerNorm rstd is
computed as exp(-0.5*ln(var+eps)) so every Activation-engine op (Identity
bias-add, Exp for ELU, Ln/Exp for rstd, Copy) lives in ONE act-func table
set - the baseline burned 22us reloading tables between Exp and Sqrt.
The -mean and rstd rows are broadcast across partitions with two K=1
matmuls, then copied once to SBUF bf16 so the LN apply (add+mul) runs in
DVE 2x mode. The last layer's bias is preloaded into PSUM with a K=1
matmul so the batch-major output needs only an Act Copy before DMA; the
output DMA issues from the idle GPSIMD DGE queue and the zs/act inputs
arrive as one large rearranged DMA each instead of 20 small ones.

Sharding: pure data-parallel, batch 16384 -> 8 x 2048. Activations are kept
feature-major ([feat, batch]) on chip; the host transposes + bf16-casts the
zs/action shards. LayerNorm stats (feature = partition axis) are computed
with 1/512-scaled ones-matmuls on the PE.

n_reps>1 wraps the computation in a hardware For-loop; used only by the
local timing harness to measure per-iteration device time by wall-clock
slope.
"""

import contextlib
import sys

sys.path.insert(0, "/opt/trn_rl_repo")

import numpy as np
import ml_dtypes

import concourse.bass as bass  # noqa: E402
import concourse.tile as tile  # noqa: E402
from concourse import bacc, mybir  # noqa: E402
from concourse.bass_utils import run_bass_kernel_spmd  # noqa: E402

F32 = mybir.dt.float32
BF16 = mybir.dt.bfloat16
BF = ml_dtypes.bfloat16
AF = mybir.ActivationFunctionType
OP = mybir.AluOpType

NCORES = 8
B_LOC = 2048          # batch rows per core
BC = 512              # batch columns per chunk (psum free dim)
NB = B_LOC // BC      # 4 batch chunks
P = 128
LN_EPS = 1e-5


def build_module(n_reps=1):
    nc = bacc.Bacc("TRN2", target_bir_lowering=False, debug=False,
                   enable_asserts=False, num_devices=NCORES)

    def din(name, shape, dt=BF16):
        return nc.dram_tensor(name, list(shape), dt, kind="ExternalInput").ap()

    zsT = din("zsT", (512, B_LOC))
    actT = din("actT", (8, B_LOC))
    wza = din("wza", (8, 2, 256))
    w1 = din("w1", (1536, 512))
    w2 = din("w2", (1024, 512))
    w3 = din("w3", (1024, 512))
    bza_p = din("bza_p", (128, 2), F32)
    b1_p = din("b1_p", (128, 4), F32)
    b2_p = din("b2_p", (128, 4), F32)
    b3_r = din("b3_r", (1, 512))
    ones_c = din("ones_c", (1, 128))
    oinv_m = din("oinv_m", (128, 1))
    out = nc.dram_tensor("out", [B_LOC, 512], F32, kind="ExternalOutput").ap()

    with tile.TileContext(nc) as tc:
        with (
            tc.tile_pool(name="wpool", bufs=1) as wp,
            tc.tile_pool(name="io", bufs=1) as io,
            tc.tile_pool(name="uvza", bufs=2) as uvza,
            tc.tile_pool(name="uvzs", bufs=3) as uvzs,
            tc.tile_pool(name="uv1", bufs=2) as uv1,
            tc.tile_pool(name="uv2", bufs=2) as uv2,
            tc.tile_pool(name="zcb", bufs=3) as zcbp,
            tc.tile_pool(name="eph", bufs=2) as eph,
            tc.tile_pool(name="rows", bufs=2) as rows,
            tc.tile_pool(name="bcsb", bufs=2) as bcsb,
            tc.tile_pool(name="osb", bufs=2) as osbp,
            tc.tile_pool(name="psz", bufs=5, space="PSUM") as psz,
            tc.tile_pool(name="psst", bufs=1, space="PSUM") as psst,
            tc.tile_pool(name="psbc", bufs=1, space="PSUM") as psbc,
        ):
            # ---- persistent weights / constants ----
            w1_sb = wp.tile([P, 12, 512], BF16)
            nc.sync.dma_start(w1_sb[:], w1.rearrange("(c p) o -> p c o", p=P))
            w2_sb = wp.tile([P, 8, 512], BF16)
            nc.sync.dma_start(w2_sb[:], w2.rearrange("(c p) o -> p c o", p=P))
            w3_sb = wp.tile([P, 8, 512], BF16)
            nc.sync.dma_start(w3_sb[:], w3.rearrange("(c p) o -> p c o", p=P))
            wza_sb = wp.tile([8, 2, 256], BF16)
            nc.sync.dma_start(wza_sb[:], wza[:, :, :])
            bza_sb = wp.tile([P, 2], F32)
            nc.sync.dma_start(bza_sb[:], bza_p[:, :])
            b1_sb = wp.tile([P, 4], F32)
            nc.sync.dma_start(b1_sb[:], b1_p[:, :])
            b2_sb = wp.tile([P, 4], F32)
            nc.sync.dma_start(b2_sb[:], b2_p[:, :])
            b3_sb = wp.tile([1, 512], BF16)
            nc.sync.dma_start(b3_sb[:], b3_r[:, :])
            ones_col = wp.tile([1, 128], BF16)
            nc.sync.dma_start(ones_col[:], ones_c[:, :])
            oinv_mcol = wp.tile([P, 1], BF16)  # 1/512 -> stats matmuls -> means
            nc.sync.dma_start(oinv_mcol[:], oinv_m[:, :])
            eps_sb = wp.tile([1, 1], F32)
            nc.vector.memset(eps_sb[:], LN_EPS)

            def body():
                # ---- per-chunk persistent activation tiles (bf16) ----
                upza = [uvza.tile([P, 2, BC], BF16, tag="upza", name=f"upza{b}")
                        for b in range(NB)]
                vza = [uvza.tile([P, 2, BC], BF16, tag="vza", name=f"vza{b}")
                       for b in range(NB)]
                up1 = [uv1.tile([P, 4, BC], BF16, tag="up1", name=f"up1{b}")
                       for b in range(NB)]
                v1 = [uv1.tile([P, 4, BC], BF16, tag="v1", name=f"v1{b}")
                      for b in range(NB)]
                up2 = [uv2.tile([P, 4, BC], BF16, tag="up2", name=f"up2{b}")
                       for b in range(NB)]
                v2 = [uv2.tile([P, 4, BC], BF16, tag="v2", name=f"v2{b}")
                      for b in range(NB)]

                # ==== consolidated input DMA + clips (single big transfers)
                act_raw = eph.tile([8, B_LOC], BF16, tag="act_raw", bufs=1)
                nc.sync.dma_start(act_raw[:], actT[:, :])
                u_act = eph.tile([8, B_LOC], BF16, tag="u_act", bufs=1)
                nc.vector.tensor_scalar(u_act[:], act_raw[:],
                                        -1.0, 1.0, OP.max, OP.min)
                v_act = eph.tile([8, B_LOC], BF16, tag="v_act", bufs=1)
                nc.vector.tensor_scalar(v_act[:], act_raw[:],
                                        0.0, 1.0, OP.max, OP.min)
                zsraw = io.tile([P, 4, B_LOC], BF16, tag="zsraw")
                nc.sync.dma_start(zsraw[:],
                                  zsT.rearrange("(c p) b -> p c b", p=P))
                uzs, vzs = [], []
                for b in range(NB):
                    bs = slice(b * BC, (b + 1) * BC)
                    u = uvzs.tile([P, 4, BC], BF16, tag="uzs", name=f"uzs{b}")
                    nc.vector.tensor_scalar(u[:], zsraw[:, :, bs],
                                            -1.0, 1.0, OP.max, OP.min)
                    v = uvzs.tile([P, 4, BC], BF16, tag="vzs", name=f"vzs{b}")
                    nc.vector.tensor_scalar(v[:], zsraw[:, :, bs],
                                            0.0, 1.0, OP.max, OP.min)
                    uzs.append(u)
                    vzs.append(v)

                # ==== l0: za = (pre-elu) APL(action) ====
                for b in range(NB):
                    bs = slice(b * BC, (b + 1) * BC)
                    zb = zcbp.tile([P, 2, BC], BF16, tag="zb_za")
                    for o in range(2):
                        zps = psz.tile([P, BC], F32, tag="z")
                        nc.tensor.matmul(zps[:], wza_sb[:, 0, bass.ts(o, 128)],
                                         u_act[:, bs], start=True, stop=False)
                        nc.tensor.matmul(zps[:], wza_sb[:, 1, bass.ts(o, 128)],
                                         v_act[:, bs], start=False, stop=True)
                        nc.scalar.activation(zb[:, o, :], zps[:], AF.Identity,
                                             bias=bza_sb[:, o:o + 1])
                    nc.vector.tensor_scalar(vza[b][:], zb[:], 0.0, 1.0,
                                            OP.max, OP.min)
                    nmin = eph.tile([P, 2, BC], BF16, tag="nmin_za")
                    nc.vector.tensor_scalar(nmin[:], zb[:], 0.0, None, OP.min)
                    nc.scalar.activation(nmin[:], nmin[:], AF.Exp)
                    nc.vector.tensor_add(upza[b][:], vza[b][:], nmin[:])

                # ==== hidden layers l1, l2 ====
                def hidden_layer(b, KC, w_sb, b_sb, rhs_fn, up_dst, v_dst):
                    zc = zcbp.tile([P, 4, BC], BF16, tag="zcb")
                    for o in range(4):
                        zps = psz.tile([P, BC], F32, tag="z")
                        for k in range(KC):
                            nc.tensor.matmul(zps[:],
                                             w_sb[:, k, bass.ts(o, 128)],
                                             rhs_fn(k),
                                             start=(k == 0),
                                             stop=(k == KC - 1))
                        nc.scalar.activation(zc[:, o, :], zps[:],
                                             AF.Identity,
                                             bias=b_sb[:, o:o + 1])
                    # stats: mean and mean-square via 1/512-ones matmuls (M=1)
                    st_ps = psst.tile([33, BC], F32, tag="st")
                    for o in range(4):
                        nc.tensor.matmul(st_ps[0:1, :], oinv_mcol[:],
                                         zc[:, o, :],
                                         start=(o == 0), stop=(o == 3))
                    zsq = eph.tile([P, 4, BC], BF16, tag="zsq")
                    nc.vector.tensor_mul(zsq[:], zc[:], zc[:])
                    for o in range(4):
                        nc.tensor.matmul(st_ps[32:33, :], oinv_mcol[:],
                                         zsq[:, o, :],
                                         start=(o == 0), stop=(o == 3))
                    # rows: cat = [-mean | rstd] as one [1, 2*BC] bf16 row
                    cat = rows.tile([1, 2, BC], BF16, tag="cat")
                    nc.vector.tensor_scalar(cat[0:1, 0, :], st_ps[0:1, :],
                                            -1.0, None, OP.mult)
                    m2 = rows.tile([1, BC], F32, tag="scr", name="m2")
                    nc.scalar.activation(m2[:], st_ps[0:1, :], AF.Square)
                    var = rows.tile([1, BC], F32, tag="scr", name="var")
                    nc.vector.tensor_sub(var[:], st_ps[32:33, :], m2[:])
                    lnv = rows.tile([1, BC], F32, tag="scr", name="lnv")
                    nc.scalar.activation(lnv[:], var[:], AF.Ln, bias=eps_sb[:])
                    with nc.allow_low_precision(
                            reason="rstd rounds to bf16 for PE broadcast"):
                        nc.scalar.activation(cat[0:1, 1, :], lnv[:], AF.Exp,
                                             scale=-0.5)
                    # broadcast [-m | s] across partitions (one K=1 matmul)
                    bc_ps = psbc.tile([P, 2, BC], F32, tag="bc")
                    nc.tensor.matmul(bc_ps[:, 0, :], ones_col[:],
                                     cat[0:1, 0, :], start=True, stop=True)
                    nc.tensor.matmul(bc_ps[:, 1, :], ones_col[:],
                                     cat[0:1, 1, :], start=True, stop=True)
                    bc_sb = bcsb.tile([P, 2, BC], BF16, tag="bc_sb")
                    nc.scalar.activation(bc_sb[:], bc_ps[:], AF.Copy)
                    mb_b = bc_sb[:, 0:1, :].to_broadcast([P, 4, BC])
                    sb_b = bc_sb[:, 1:2, :].to_broadcast([P, 4, BC])
                    h = zc
                    nc.vector.tensor_add(h[:], h[:], mb_b)
                    nc.vector.tensor_mul(h[:], h[:], sb_b)
                    nc.vector.tensor_scalar(v_dst[:], h[:],
                                            0.0, 1.0, OP.max, OP.min)
                    nmin = eph.tile([P, 4, BC], BF16, tag="nmin")
                    nc.vector.tensor_scalar(nmin[:], h[:], 0.0, None, OP.min)
                    nc.scalar.activation(nmin[:], nmin[:], AF.Exp)
                    nc.vector.tensor_add(up_dst[:], v_dst[:], nmin[:])

                for b in range(NB):
                    def rhs1(k, b=b):
                        if k < 4:
                            return uzs[b][:, k, :]
                        if k < 8:
                            return vzs[b][:, k - 4, :]
                        if k < 10:
                            return upza[b][:, k - 8, :]
                        return vza[b][:, k - 10, :]

                    hidden_layer(b, 12, w1_sb, b1_sb, rhs1, up1[b], v1[b])

                for b in range(NB):
                    def rhs2(k, b=b):
                        return up1[b][:, k, :] if k < 4 else v1[b][:, k - 4, :]

                    hidden_layer(b, 8, w2_sb, b2_sb, rhs2, up2[b], v2[b])

                # ==== l3: batch-major out, bias preloaded via K=1 matmul ====
                for b in range(NB):
                    for h in range(2):
                        osb = osbp.tile([P, 2, 512], F32, tag="osb")
                        for qq in range(2):
                            q = h * 2 + qq
                            qs = bass.ts(q, 128)
                            ops = psz.tile([P, 512], F32, tag="z")
                            nc.tensor.matmul(ops[:], ones_col[:], b3_sb[:],
                                             start=True, stop=False)
                            for k in range(8):
                                lhsT = (up2[b][:, k, qs] if k < 4
                                        else v2[b][:, k - 4, qs])
                                nc.tensor.matmul(ops[:], lhsT, w3_sb[:, k, :],
                                                 start=False, stop=(k == 7))
                            nc.scalar.activation(osb[:, qq, :], ops[:],
                                                 AF.Copy)
                        nc.gpsimd.dma_start(
                            out[b * BC + h * 256:
                                b * BC + (h + 1) * 256, :].rearrange(
                                "(q p) n -> p q n", p=P),
                            osb[:])

            rep_ctx = (tc.For_i(0, n_reps, 1) if n_reps > 1
                       else contextlib.nullcontext())
            with rep_ctx:
                body()

    # The act-table pass picks the FIRST set containing each function, so
    # Ln (set 5) and Exp (set 0) alternate and every hidden-layer chunk
    # pays two 1283ns table loads.  All functions this kernel uses
    # (Identity, Square, Ln, Exp, Copy) live together in
    # 'natural_log_exp_and_others' (set 6): censor sets 0-5 in the copy the
    # pass sees so every function first-matches at set 6 -> ONE load total.
    # Only the pass's choice is patched; emitted ids still index the real
    # act_info.json list, and set 6's real contents cover every function.
    import concourse.bacc as _bacc_mod
    _orig_tables = _bacc_mod.get_activation_tables
    _used = {AF.Identity, AF.Square, AF.Ln, AF.Exp, AF.Copy}

    def _censored_tables(arch):
        tabs = dict(_orig_tables(arch))
        names = list(tabs)
        for name in names[:6]:
            tabs[name] = tabs[name] - _used
        return tabs

    _bacc_mod.get_activation_tables = _censored_tables
    try:
        nc.compile()
    finally:
        _bacc_mod.get_activation_tables = _orig_tables
    return nc


def fold_weights(W_za, W1, W2, W3):
    def fold(vals):
        V = vals.astype(np.float64)
        A = V[:, :, 1] - V[:, :, 0]
        C = V[:, :, 0] + V[:, :, 2] - 2.0 * V[:, :, 1]
        b = V[:, :, 1].sum(axis=0)
        return A, C, b

    A0, C0, b0 = fold(W_za)
    A1, C1, b1 = fold(W1)
    A2, C2, b2 = fold(W2)
    A3, C3, b3 = fold(W3)

    wza = np.stack([A0, C0], axis=1)                             # [8, 2, 256]
    w1 = np.concatenate([A1[:512], C1[:512], A1[512:], C1[512:]], axis=0)
    w2 = np.concatenate([A2, C2], axis=0)                        # [1024, 512]
    w3 = np.concatenate([A3, C3], axis=0)                        # [1024, 512]
    b1e = b1 - A1[512:].sum(axis=0)      # za u' carries +1 shift
    b2e = b2 - A2.sum(axis=0)
    b3e = b3 - A3.sum(axis=0)

    f = np.float32
    return {
        "wza": np.ascontiguousarray(wza.astype(f), BF),
        "w1": np.ascontiguousarray(w1.astype(f), BF),
        "w2": np.ascontiguousarray(w2.astype(f), BF),
        "w3": np.ascontiguousarray(w3.astype(f), BF),
        "bza_p": np.ascontiguousarray(b0.reshape(2, 128).T, f),
        "b1_p": np.ascontiguousarray(b1e.reshape(4, 128).T, f),
        "b2_p": np.ascontiguousarray(b2e.reshape(4, 128).T, f),
        "b3_r": np.ascontiguousarray(b3e.reshape(1, 512).astype(f), BF),
        "ones_c": np.ones((1, 128), BF),
        "oinv_m": np.full((128, 1), 1.0 / 512.0, BF),
    }


_NC_CACHE = {}


def get_module(n_reps=1):
    key = f"nc{n_reps}"
    if key not in _NC_CACHE:
        _NC_CACHE[key] = build_module(n_reps)
    return _NC_CACHE[key]


def make_in_maps(zs, action, W_za, W1, W2, W3):
    wmap = fold_weights(np.asarray(W_za), np.asarray(W1), np.asarray(W2),
                        np.asarray(W3))
    in_maps = []
    for c in range(NCORES):
        sl = slice(c * B_LOC, (c + 1) * B_LOC)
        m = dict(wmap)
        m["zsT"] = np.ascontiguousarray(np.asarray(zs)[sl].T.astype(np.float32),
                                        BF)
        m["actT"] = np.ascontiguousarray(
            np.asarray(action)[sl].T.astype(np.float32), BF)
        in_maps.append(m)
    return in_maps


def kernel(zs, action, W_za, W1, W2, W3, _trace=False, _tmpdir=None):
    nc = get_module()
    in_maps = make_in_maps(zs, action, W_za, W1, W2, W3)
    res = run_bass_kernel_spmd(nc, in_maps, core_ids=list(range(NCORES)),
                               trace=_trace, tmpdir=_tmpdir)
    out = np.concatenate([res.results[c]["out"] for c in range(NCORES)],
                         axis=0).astype(np.float32)
    if _trace:
        kernel.last_exec_time_ns = res.exec_time_ns
        kernel.last_results = res
    return out


# revision 20
# speedup vs baseline: 1.0817x; 1.0064x over previous
"""Trainium2 Bass kernel for nn_Encoder (KAN-style piecewise-linear MLP encoder).

Math: each adaptive piecewise-linear layer (P=3 knots on [-1,1]) collapses to
    out = u @ A + v @ C + bias,   u = clip(x,-1,1), v = clip(x,0,1)
with A = V1-V0, C = V0+V2-2*V1, bias = colsum(V1)  (hat basis sums to 1).
ELU never needs materializing: the next layer only consumes
    v' = clip(elu(h),0,1) = clip(h,0,1)
    u' = clip(elu(h),-1,1) + 1 = v' + exp(min(h,0))
and the +1 shift is folded into the next layer's bias (bias -= colsum(A_rows)).

v2 (this file): all activations/weights in bf16 (PE rate is unchanged vs
float32r, but DVE elementwise ops hit the 2x/4x fast modes and SBUF tiles
halve, which buys enough space to keep all 4 batch chunks in flight so the
tensor engine never drops out of its high p-state). LayerNorm rstd is
computed as exp(-0.5*ln(var+eps)) so every Activation-engine op (Identity
bias-add, Exp for ELU, Ln/Exp for rstd, Copy) lives in ONE act-func table
set - the baseline burned 22us reloading tables between Exp and Sqrt.
The -mean and rstd rows are broadcast across partitions with two K=1
matmuls, then copied once to SBUF bf16 so the LN apply (add+mul) runs in
DVE 2x mode. The last layer's bias is preloaded into PSUM with a K=1
matmul so the batch-major output needs only an Act Copy before DMA; the
output DMA issues from the idle GPSIMD DGE queue and the zs/act inputs
arrive as one large rearranged DMA each instead of 20 small ones.

Sharding: pure data-parallel, batch 16384 -> 8 x 2048. Activations are kept
feature-major ([feat, batch]) on chip; the host transposes + bf16-casts the
zs/action shards. LayerNorm stats (feature = partition axis) are computed
with 1/512-scaled ones-matmuls on the PE.

n_reps>1 wraps the computation in a hardware For-loop; used only by the
local timing harness to measure per-iteration device time by wall-clock
slope.
"""

import contextlib
import sys

sys.path.insert(0, "/opt/trn_rl_repo")

import numpy as np
import ml_dtypes

import concourse.bass as bass  # noqa: E402
import concourse.tile as tile  # noqa: E402
from concourse import bacc, mybir  # noqa: E402
from concourse.bass_utils import run_bass_kernel_spmd  # noqa: E402

F32 = mybir.dt.float32
BF16 = mybir.dt.bfloat16
BF = ml_dtypes.bfloat16
AF = mybir.ActivationFunctionType
OP = mybir.AluOpType

NCORES = 8
B_LOC = 2048          # batch rows per core
BC = 512              # batch columns per chunk (psum free dim)
NB = B_LOC // BC      # 4 batch chunks
P = 128
LN_EPS = 1e-5


def build_module(n_reps=1):
    nc = bacc.Bacc("TRN2", target_bir_lowering=False, debug=False,
                   enable_asserts=False, num_devices=NCORES)

    def din(name, shape, dt=BF16):
        return nc.dram_tensor(name, list(shape), dt, kind="ExternalInput").ap()

    zsT = din("zsT", (512, B_LOC))
    actT = din("actT", (8, B_LOC))
    wza = din("wza", (8, 2, 256))
    w1 = din("w1", (1536, 512))
    w2 = din("w2", (1024, 512))
    w3 = din("w3", (1024, 512))
    bza_p = din("bza_p", (128, 2), F32)
    b1_p = din("b1_p", (128, 4), F32)
    b2_p = din("b2_p", (128, 4), F32)
    b3_r = din("b3_r", (1, 512))
    ones_c = din("ones_c", (1, 128))
    oinv_m = din("oinv_m", (128, 1))
    out = nc.dram_tensor("out", [B_LOC, 512], F32, kind="ExternalOutput").ap()

    with tile.TileContext(nc) as tc:
        with (
            tc.tile_pool(name="wpool", bufs=1) as wp,
            tc.tile_pool(name="io", bufs=1) as io,
            tc.tile_pool(name="uvza", bufs=2) as uvza,
            tc.tile_pool(name="uvzs", bufs=3) as uvzs,
            tc.tile_pool(name="uv1", bufs=2) as uv1,
            tc.tile_pool(name="uv2", bufs=2) as uv2,
            tc.tile_pool(name="zcb", bufs=3) as zcbp,
            tc.tile_pool(name="eph", bufs=2) as eph,
            tc.tile_pool(name="rows", bufs=2) as rows,
            tc.tile_pool(name="bcsb", bufs=2) as bcsb,
            tc.tile_pool(name="osb", bufs=2) as osbp,
            tc.tile_pool(name="psz", bufs=5, space="PSUM") as psz,
            tc.tile_pool(name="psst", bufs=1, space="PSUM") as psst,
            tc.tile_pool(name="psbc", bufs=1, space="PSUM") as psbc,
        ):
            # ---- persistent weights / constants ----
            w1_sba = wp.tile([P, 6, 512], BF16)
            nc.sync.dma_start(w1_sba[:],
                              w1[0:768, :].rearrange("(c p) o -> p c o", p=P))
            w1_sbb = wp.tile([P, 6, 512], BF16)
            nc.sync.dma_start(w1_sbb[:],
                              w1[768:1536, :].rearrange("(c p) o -> p c o",
                                                        p=P))
            w2_sb = wp.tile([P, 8, 512], BF16)
            nc.sync.dma_start(w2_sb[:], w2.rearrange("(c p) o -> p c o", p=P))
            w3_sb = wp.tile([P, 8, 512], BF16)
            nc.sync.dma_start(w3_sb[:], w3.rearrange("(c p) o -> p c o", p=P))
            wza_sb = wp.tile([8, 2, 256], BF16)
            nc.sync.dma_start(wza_sb[:], wza[:, :, :])
            bza_sb = wp.tile([P, 2], F32)
            nc.sync.dma_start(bza_sb[:], bza_p[:, :])
            b1_sb = wp.tile([P, 4], F32)
            nc.sync.dma_start(b1_sb[:], b1_p[:, :])
            b2_sb = wp.tile([P, 4], F32)
            nc.sync.dma_start(b2_sb[:], b2_p[:, :])
            b3_sb = wp.tile([1, 512], BF16)
            nc.sync.dma_start(b3_sb[:], b3_r[:, :])
            ones_col = wp.tile([1, 128], BF16)
            nc.sync.dma_start(ones_col[:], ones_c[:, :])
            oinv_mcol = wp.tile([P, 1], BF16)  # 1/512 -> stats matmuls -> means
            nc.sync.dma_start(oinv_mcol[:], oinv_m[:, :])
            eps_sb = wp.tile([1, 1], F32)
            nc.vector.memset(eps_sb[:], LN_EPS)

            def body():
                # ---- per-chunk persistent activation tiles (bf16) ----
                upza = [uvza.tile([P, 2, BC], BF16, tag="upza", name=f"upza{b}")
                        for b in range(NB)]
                vza = [uvza.tile([P, 2, BC], BF16, tag="vza", name=f"vza{b}")
                       for b in range(NB)]
                up1 = [uv1.tile([P, 4, BC], BF16, tag="up1", name=f"up1{b}")
                       for b in range(NB)]
                v1 = [uv1.tile([P, 4, BC], BF16, tag="v1", name=f"v1{b}")
                      for b in range(NB)]
                up2 = [uv2.tile([P, 4, BC], BF16, tag="up2", name=f"up2{b}")
                       for b in range(NB)]
                v2 = [uv2.tile([P, 4, BC], BF16, tag="v2", name=f"v2{b}")
                      for b in range(NB)]

                # ==== consolidated input DMA + clips (single big transfers)
                act_raw = eph.tile([8, B_LOC], BF16, tag="act_raw", bufs=1)
                nc.sync.dma_start(act_raw[:], actT[:, :])
                u_act = eph.tile([8, B_LOC], BF16, tag="u_act", bufs=1)
                nc.vector.tensor_scalar(u_act[:], act_raw[:],
                                        -1.0, 1.0, OP.max, OP.min)
                v_act = eph.tile([8, B_LOC], BF16, tag="v_act", bufs=1)
                nc.vector.tensor_scalar(v_act[:], act_raw[:],
                                        0.0, 1.0, OP.max, OP.min)
                half = B_LOC // 2
                zsr = []
                for hh in range(2):
                    zr = io.tile([P, 4, half], BF16, tag=f"zsraw{hh}",
                                 name=f"zsraw{hh}")
                    nc.sync.dma_start(
                        zr[:], zsT[:, hh * half:(hh + 1) * half].rearrange(
                            "(c p) b -> p c b", p=P))
                    zsr.append(zr)
                uzs, vzs = [], []
                for b in range(NB):
                    zr = zsr[b // 2]
                    bs = slice((b % 2) * BC, (b % 2 + 1) * BC)
                    u = uvzs.tile([P, 4, BC], BF16, tag="uzs", name=f"uzs{b}")
                    nc.vector.tensor_scalar(u[:], zr[:, :, bs],
                                            -1.0, 1.0, OP.max, OP.min)
                    v = uvzs.tile([P, 4, BC], BF16, tag="vzs", name=f"vzs{b}")
                    nc.vector.tensor_scalar(v[:], zr[:, :, bs],
                                            0.0, 1.0, OP.max, OP.min)
                    uzs.append(u)
                    vzs.append(v)

                # ==== l0: za = (pre-elu) APL(action) ====
                for b in range(NB):
                    bs = slice(b * BC, (b + 1) * BC)
                    zb = zcbp.tile([P, 2, BC], BF16, tag="zb_za")
                    for o in range(2):
                        zps = psz.tile([P, BC], F32, tag="z")
                        nc.tensor.matmul(zps[:], wza_sb[:, 0, bass.ts(o, 128)],
                                         u_act[:, bs], start=True, stop=False)
                        nc.tensor.matmul(zps[:], wza_sb[:, 1, bass.ts(o, 128)],
                                         v_act[:, bs], start=False, stop=True)
                        nc.scalar.activation(zb[:, o, :], zps[:], AF.Identity,
                                             bias=bza_sb[:, o:o + 1])
                    nc.vector.tensor_scalar(vza[b][:], zb[:], 0.0, 1.0,
                                            OP.max, OP.min)
                    nmin = eph.tile([P, 2, BC], BF16, tag="nmin_za")
                    nc.vector.tensor_scalar(nmin[:], zb[:], 0.0, None, OP.min)
                    nc.scalar.activation(nmin[:], nmin[:], AF.Exp)
                    nc.vector.tensor_add(upza[b][:], vza[b][:], nmin[:])

                # ==== hidden layers l1, l2 ====
                def hidden_layer(b, KC, w_fn, b_sb, rhs_fn, up_dst, v_dst):
                    zc = zcbp.tile([P, 4, BC], BF16, tag="zcb")
                    for o in range(4):
                        zps = psz.tile([P, BC], F32, tag="z")
                        for k in range(KC):
                            nc.tensor.matmul(zps[:],
                                             w_fn(k, o),
                                             rhs_fn(k),
                                             start=(k == 0),
                                             stop=(k == KC - 1))
                        nc.scalar.activation(zc[:, o, :], zps[:],
                                             AF.Identity,
                                             bias=b_sb[:, o:o + 1])
                    # stats: mean and mean-square via 1/512-ones matmuls (M=1)
                    st_ps = psst.tile([33, BC], F32, tag="st")
                    for o in range(4):
                        nc.tensor.matmul(st_ps[0:1, :], oinv_mcol[:],
                                         zc[:, o, :],
                                         start=(o == 0), stop=(o == 3))
                    zsq = eph.tile([P, 4, BC], BF16, tag="zsq")
                    nc.vector.tensor_mul(zsq[:], zc[:], zc[:])
                    for o in range(4):
                        nc.tensor.matmul(st_ps[32:33, :], oinv_mcol[:],
                                         zsq[:, o, :],
                                         start=(o == 0), stop=(o == 3))
                    # rows: cat = [-mean | rstd] as one [1, 2*BC] bf16 row
                    cat = rows.tile([1, 2, BC], BF16, tag="cat")
                    nc.vector.tensor_scalar(cat[0:1, 0, :], st_ps[0:1, :],
                                            -1.0, None, OP.mult)
                    m2 = rows.tile([1, BC], F32, tag="scr", name="m2")
                    nc.scalar.activation(m2[:], st_ps[0:1, :], AF.Square)
                    var = rows.tile([1, BC], F32, tag="scr", name="var")
                    nc.vector.tensor_sub(var[:], st_ps[32:33, :], m2[:])
                    lnv = rows.tile([1, BC], F32, tag="scr", name="lnv")
                    nc.scalar.activation(lnv[:], var[:], AF.Ln, bias=eps_sb[:])
                    with nc.allow_low_precision(
                            reason="rstd rounds to bf16 for PE broadcast"):
                        nc.scalar.activation(cat[0:1, 1, :], lnv[:], AF.Exp,
                                             scale=-0.5)
                    # broadcast [-m | s] across partitions (one K=1 matmul)
                    bc_ps = psbc.tile([P, 2, BC], F32, tag="bc")
                    nc.tensor.matmul(bc_ps[:, 0, :], ones_col[:],
                                     cat[0:1, 0, :], start=True, stop=True)
                    nc.tensor.matmul(bc_ps[:, 1, :], ones_col[:],
                                     cat[0:1, 1, :], start=True, stop=True)
                    bc_sb = bcsb.tile([P, 2, BC], BF16, tag="bc_sb")
                    nc.scalar.activation(bc_sb[:], bc_ps[:], AF.Copy)
                    mb_b = bc_sb[:, 0:1, :].to_broadcast([P, 4, BC])
                    sb_b = bc_sb[:, 1:2, :].to_broadcast([P, 4, BC])
                    h = zc
                    nc.vector.tensor_add(h[:], h[:], mb_b)
                    nc.vector.tensor_mul(h[:], h[:], sb_b)
                    nc.vector.tensor_scalar(v_dst[:], h[:],
                                            0.0, 1.0, OP.max, OP.min)
                    nmin = eph.tile([P, 4, BC], BF16, tag="nmin")
                    nc.vector.tensor_scalar(nmin[:], h[:], 0.0, None, OP.min)
                    nc.scalar.activation(nmin[:], nmin[:], AF.Exp)
                    nc.vector.tensor_add(up_dst[:], v_dst[:], nmin[:])

                for b in range(NB):
                    def rhs1(k, b=b):
                        if k < 4:
                            return uzs[b][:, k, :]
                        if k < 8:
                            return vzs[b][:, k - 4, :]
                        if k < 10:
                            return upza[b][:, k - 8, :]
                        return vza[b][:, k - 10, :]

                    hidden_layer(
                        b, 12,
                        lambda k, o: (w1_sba[:, k, bass.ts(o, 128)] if k < 6
                                      else w1_sbb[:, k - 6, bass.ts(o, 128)]),
                        b1_sb, rhs1, up1[b], v1[b])

                for b in range(NB):
                    def rhs2(k, b=b):
                        return up1[b][:, k, :] if k < 4 else v1[b][:, k - 4, :]

                    hidden_layer(
                        b, 8,
                        lambda k, o: w2_sb[:, k, bass.ts(o, 128)],
                        b2_sb, rhs2, up2[b], v2[b])

                # ==== l3: batch-major out, bias preloaded via K=1 matmul ====
                for b in range(NB):
                    for h in range(2):
                        osb = osbp.tile([P, 2, 512], F32, tag="osb")
                        for qq in range(2):
                            q = h * 2 + qq
                            qs = bass.ts(q, 128)
                            ops = psz.tile([P, 512], F32, tag="z")
                            nc.tensor.matmul(ops[:], ones_col[:], b3_sb[:],
                                             start=True, stop=False)
                            for k in range(8):
                                lhsT = (up2[b][:, k, qs] if k < 4
                                        else v2[b][:, k - 4, qs])
                                nc.tensor.matmul(ops[:], lhsT, w3_sb[:, k, :],
                                                 start=False, stop=(k == 7))
                            nc.scalar.activation(osb[:, qq, :], ops[:],
                                                 AF.Copy)
                        nc.gpsimd.dma_start(
                            out[b * BC + h * 256:
                                b * BC + (h + 1) * 256, :].rearrange(
                                "(q p) n -> p q n", p=P),
                            osb[:])

            rep_ctx = (tc.For_i(0, n_reps, 1) if n_reps > 1
                       else contextlib.nullcontext())
            with rep_ctx:
                body()

    # The act-table pass picks the FIRST set containing each function, so
    # Ln (set 5) and Exp (set 0) alternate and every hidden-layer chunk
    # pays two 1283ns table loads.  All functions this kernel uses
    # (Identity, Square, Ln, Exp, Copy) live together in
    # 'natural_log_exp_and_others' (set 6): censor sets 0-5 in the copy the
    # pass sees so every function first-matches at set 6 -> ONE load total.
    # Only the pass's choice is patched; emitted ids still index the real
    # act_info.json list, and set 6's real contents cover every function.
    import concourse.bacc as _bacc_mod
    _orig_tables = _bacc_mod.get_activation_tables
    _used = {AF.Identity, AF.Square, AF.Ln, AF.Exp, AF.Copy}

    def _censored_tables(arch):
        tabs = dict(_orig_tables(arch))
        names = list(tabs)
        for name in names[:6]:
            tabs[name] = tabs[name] - _used
        return tabs

    _bacc_mod.get_activation_tables = _censored_tables
    try:
        nc.compile()
    finally:
        _bacc_mod.get_activation_tables = _orig_tables
    return nc


def fold_weights(W_za, W1, W2, W3):
    def fold(vals):
        V = vals.astype(np.float64)
        A = V[:, :, 1] - V[:, :, 0]
        C = V[:, :, 0] + V[:, :, 2] - 2.0 * V[:, :, 1]
        b = V[:, :, 1].sum(axis=0)
        return A, C, b

    A0, C0, b0 = fold(W_za)
    A1, C1, b1 = fold(W1)
    A2, C2, b2 = fold(W2)
    A3, C3, b3 = fold(W3)

    wza = np.stack([A0, C0], axis=1)                             # [8, 2, 256]
    w1 = np.concatenate([A1[:512], C1[:512], A1[512:], C1[512:]], axis=0)
    w2 = np.concatenate([A2, C2], axis=0)                        # [1024, 512]
    w3 = np.concatenate([A3, C3], axis=0)                        # [1024, 512]
    b1e = b1 - A1[512:].sum(axis=0)      # za u' carries +1 shift
    b2e = b2 - A2.sum(axis=0)
    b3e = b3 - A3.sum(axis=0)

    f = np.float32
    return {
        "wza": np.ascontiguousarray(wza.astype(f), BF),
        "w1": np.ascontiguousarray(w1.astype(f), BF),
        "w2": np.ascontiguousarray(w2.astype(f), BF),
        "w3": np.ascontiguousarray(w3.astype(f), BF),
        "bza_p": np.ascontiguousarray(b0.reshape(2, 128).T, f),
        "b1_p": np.ascontiguousarray(b1e.reshape(4, 128).T, f),
        "b2_p": np.ascontiguousarray(b2e.reshape(4, 128).T, f),
        "b3_r": np.ascontiguousarray(b3e.reshape(1, 512).astype(f), BF),
        "ones_c": np.ones((1, 128), BF),
        "oinv_m": np.full((128, 1), 1.0 / 512.0, BF),
    }


_NC_CACHE = {}


def get_module(n_reps=1):
    key = f"nc{n_reps}"
    if key not in _NC_CACHE:
        _NC_CACHE[key] = build_module(n_reps)
    return _NC_CACHE[key]


def make_in_maps(zs, action, W_za, W1, W2, W3):
    wmap = fold_weights(np.asarray(W_za), np.asarray(W1), np.asarray(W2),
                        np.asarray(W3))
    in_maps = []
    for c in range(NCORES):
        sl = slice(c * B_LOC, (c + 1) * B_LOC)
        m = dict(wmap)
        m["zsT"] = np.ascontiguousarray(np.asarray(zs)[sl].T.astype(np.float32),
                                        BF)
        m["actT"] = np.ascontiguousarray(
            np.asarray(action)[sl].T.astype(np.float32), BF)
        in_maps.append(m)
    return in_maps


def kernel(zs, action, W_za, W1, W2, W3, _trace=False, _tmpdir=None):
    nc = get_module()
    in_maps = make_in_maps(zs, action, W_za, W1, W2, W3)
    res = run_bass_kernel_spmd(nc, in_maps, core_ids=list(range(NCORES)),
                               trace=_trace, tmpdir=_tmpdir)
    out = np.concatenate([res.results[c]["out"] for c in range(NCORES)],
                         axis=0).astype(np.float32)
    if _trace:
        kernel.last_exec_time_ns = res.exec_time_ns
        kernel.last_results = res
    return out
